# revision 9
# baseline (speedup 1.0000x reference)
"""Causal self-attention (weight-modulated) Trainium2 kernel, 8-core SPMD.

Reference semantics (B=2, T=2048, C=512, 8 heads, hd=64):
    v0  = x @ Wv.T + bv
    v   = v0 * w[:, :, None]            # w = weight[:, :, 0]
    att = softmax(mask((v0h @ v0h^T) * w[key] / sqrt(hd)))
    y   = att @ vh
    out = y @ Wp.T + bp

Sharding: core = (b, p) with b = batch, p = query-quarter. Each core
computes 512 contiguous query rows against all keys of its batch.
Keys are host-permuted so the causal diagonal 512-block sits at key
slot 0 for every core; the program is identical across cores (SPMD)
and per-core differences live entirely in the input data:
  - kxT   [C, T]   x^T with permuted+padded key columns
  - kbvec [T, 1]   additive exp-bias: 0 real keys, -1e30 padding
  - w8vec [T, 1]   w_perm / sqrt(hd) (exp scale; also folds wl)
  - wvec  [T, 1]   w_perm (value scaling wr)
Because wl == wr == w, the "keys" of the score matmul are just the
value projection v0 (+bv) with no w applied, and w is applied once in
the exp scale (score layout is [key, query], so w[key] is a
per-partition scalar). Queries are slot 0 of the same v0^T tensor.
The softmax denominator comes for free as a 65th ones-column in the
AV matmul's stationary operand.
"""

import numpy as np

B, T, C = 2, 2048, 512
NH, HD = 8, 64
P = 128
QB = 512                # query rows per core
NKB = 4                 # key blocks of 512
NSB = 16                # key sub-blocks of 128
NEG = -1.0e30

_cache = {}


def _split_multi_waits(nc, mybir):
    """Walrus in this container encodes at most ONE sync wait (and one
    update) per instruction; Tile's sem assignment emits several. Hoist
    excess waits onto single-wait NOPs placed just before the
    instruction on the same engine (sequencer semantics are identical:
    the engine blocks on each wait, then issues the instruction), and
    excess updates of non-DMA instructions onto NOPs just after."""
    dma_ops = {"DMACopy", "DMATranspose", "TensorCopy"}
    for f in nc.m.functions:
        for bb in f.blocks:
            new = []
            changed = False
            for inst in bb.instructions:
                si = inst.sync_info
                waits = list(si.on_wait or []) if si is not None else []
                ups = list(si.on_update or []) if si is not None else []
                is_dma = inst.concise_opcode() in dma_ops if hasattr(
                    inst, "concise_opcode") else False
                post = []
                if si is not None and len(waits) > 1:
                    for w in waits[:-1]:
                        nop = mybir.InstNoOp(
                            name=nc.get_next_instruction_name(),
                            sync_info=mybir.SyncInfo(on_wait=[w], on_update=[]),
                            bass_nofuse=True,
                            engine=inst.engine,
                        )
                        nc.register_instruction(nop, overwrite=True)
                        new.append(nop)
                    waits = waits[-1:]
                    inst.sync_info = mybir.SyncInfo(on_wait=waits, on_update=ups)
                    changed = True
                if si is not None and len(ups) > 1 and not is_dma:
                    for u in ups[1:]:
                        nop = mybir.InstNoOp(
                            name=nc.get_next_instruction_name(),
                            sync_info=mybir.SyncInfo(on_wait=[], on_update=[u]),
                            bass_nofuse=True,
                            engine=inst.engine,
                        )
                        nc.register_instruction(nop, overwrite=True)
                        post.append(nop)
                    inst.sync_info = mybir.SyncInfo(
                        on_wait=waits, on_update=ups[:1])
                    changed = True
                new.append(inst)
                new.extend(post)
            if changed:
                bb.instructions = new


def _trineg_const():
    # trineg[s, k*512 + t] = 0 where query t (local) may see key 128k+s
    # (t >= 128k+s), else NEG. Slot-0 keys are the query rows themselves.
    s = np.arange(P)[:, None]
    out = np.empty((P, NKB * QB), np.float32)
    for k in range(NKB):
        t = np.arange(QB)[None, :]
        out[:, k * QB:(k + 1) * QB] = np.where(t >= P * k + s, 0.0, NEG)
    return out


def _build_nc():
    import concourse.bass as bass
    import concourse.mybir as mybir

    from concourse.tile import TileContext
    f32 = mybir.dt.float32
    f32r = mybir.dt.float32r
    AF = mybir.ActivationFunctionType
    ALU = mybir.AluOpType

    nc = bass.Bass()

    kxT = nc.dram_tensor("kxT", [C, T], f32r, kind="ExternalInput")
    wvt = nc.dram_tensor("wvt", [C, C], f32r, kind="ExternalInput")
    wpt = nc.dram_tensor("wpt", [C, C], f32r, kind="ExternalInput")
    bvp = nc.dram_tensor("bvp", [C, 1], f32, kind="ExternalInput")
    bvr = nc.dram_tensor("bvr", [1, C], f32r, kind="ExternalInput")
    bpp = nc.dram_tensor("bpp", [C, 1], f32, kind="ExternalInput")
    wvec = nc.dram_tensor("wvec", [T, 1], f32, kind="ExternalInput")
    w8vec = nc.dram_tensor("w8vec", [T, 1], f32, kind="ExternalInput")
    kbvec = nc.dram_tensor("kbvec", [T, 1], f32, kind="ExternalInput")
    onesr = nc.dram_tensor("onesr", [1, P], f32r, kind="ExternalInput")
    onescol = nc.dram_tensor("onescol", [P, NH], f32r, kind="ExternalInput")
    outT = nc.dram_tensor("outT", [C, QB], f32, kind="ExternalOutput")

    trineg_d = nc.inline_tensor(_trineg_const(), name="trineg")

    def r(ap):
        return ap

    with TileContext(nc) as tc:
        with (
            tc.tile_pool(name="persist", bufs=1) as pp,
            tc.tile_pool(name="stream", bufs=3) as sp,
            tc.tile_pool(name="psum", bufs=2, space="PSUM") as qq,
        ):
            # ---- persistent SBUF tensors ----
            kx_sb = [pp.tile([P, T], f32r, tag=f"kx{i}", name=f"kx{i}") for i in range(4)]
            wvt_sb = [pp.tile([P, C], f32r, tag=f"wvt{i}", name=f"wvt{i}") for i in range(4)]
            wpt_sb = [pp.tile([P, C], f32r, tag=f"wpt{i}", name=f"wpt{i}") for i in range(4)]
            vT_sb = [pp.tile([P, T], f32r, tag=f"vT{i}", name=f"vT{i}") for i in range(4)]
            va_sb = [pp.tile([P, NH * (HD + 1)], f32r, tag=f"va{i}", name=f"va{i}")
                     for i in range(NSB)]
            y_sb = [pp.tile([P, QB], f32r, tag=f"y{i}", name=f"ySB{i}") for i in range(4)]
            tri_sb = pp.tile([P, NKB * QB], f32, tag="tri")
            bvp_sb = pp.tile([P, C // P], f32, tag="bvp")
            bvr_sb = pp.tile([1, C], f32r, tag="bvr")
            bpp_sb = pp.tile([P, C // P], f32, tag="bpp")
            wv_sb = pp.tile([P, NSB], f32, tag="wv")
            w8_sb = pp.tile([P, NSB], f32, tag="w8")
            kb_sb = pp.tile([P, NSB], f32, tag="kb")
            ones_sb = pp.tile([1, P], f32r, tag="ones")
            onesc_sb = pp.tile([P, NH], f32r, tag="onesc")

            for i in range(4):
                nc.sync.dma_start(out=kx_sb[i][:], in_=kxT[i * P:(i + 1) * P, :])
                nc.sync.dma_start(out=wvt_sb[i][:], in_=wvt[i * P:(i + 1) * P, :])
                nc.sync.dma_start(out=wpt_sb[i][:], in_=wpt[i * P:(i + 1) * P, :])
            nc.sync.dma_start(out=tri_sb[:], in_=trineg_d[:])
            nc.sync.dma_start(
                out=bvp_sb[:], in_=bvp.rearrange("(n p) o -> p (n o)", p=P))
            nc.sync.dma_start(out=bvr_sb[:], in_=bvr[:])
            nc.sync.dma_start(
                out=bpp_sb[:], in_=bpp.rearrange("(n p) o -> p (n o)", p=P))
            nc.sync.dma_start(
                out=wv_sb[:], in_=wvec.rearrange("(n p) o -> p (n o)", p=P))
            nc.sync.dma_start(
                out=w8_sb[:], in_=w8vec.rearrange("(n p) o -> p (n o)", p=P))
            nc.sync.dma_start(
                out=kb_sb[:], in_=kbvec.rearrange("(n p) o -> p (n o)", p=P))
            nc.sync.dma_start(out=ones_sb[:], in_=onesr[:])
            nc.sync.dma_start(out=onesc_sb[:], in_=onescol[:])

            # ---- phase A: vT = (x @ Wv.T + bv)^T  [c, s] ----
            for i in range(4):            # c' partition block
                for j in range(NKB):      # key column block
                    ps = qq.tile([P, QB], f32, tag="vps", name="vps")
                    for k in range(4):    # contraction block
                        nc.tensor.matmul(
                            ps[:],
                            r(wvt_sb[k][:, i * P:(i + 1) * P]),
                            r(kx_sb[k][:, j * QB:(j + 1) * QB]),
                            start=(k == 0), stop=(k == 3),
                        )
                    nc.vector.tensor_scalar_add(
                        vT_sb[i][:, j * QB:(j + 1) * QB], ps[:], bvp_sb[:, i:i + 1])

            # ---- phase A2: v_aug[s, 8*(64+1)] = (v0 + bv) * w[s], ones col ----
            for sb in range(NSB):
                ps = qq.tile([P, C], f32, tag="vps", name="vps")
                for k in range(4):
                    nc.tensor.matmul(
                        ps[:],
                        r(kx_sb[k][:, sb * P:(sb + 1) * P]),
                        r(wvt_sb[k][:]),
                        start=(k == 0), stop=False,
                    )
                # += ones[s] x bv  (K=1 matmul adds the free-axis bias)
                nc.tensor.matmul(
                    ps[:], r(ones_sb[:]), r(bvr_sb[:]), start=False, stop=True)
                va3 = va_sb[sb].rearrange("p (h d) -> p h d", d=HD + 1)
                nc.vector.tensor_scalar_mul(
                    va3[:, :, 0:HD],
                    ps[:].rearrange("p (h d) -> p h d", d=HD),
                    wv_sb[:, sb:sb + 1],
                )
                nc.vector.tensor_copy(
                    va3[:, :, HD:HD + 1],
                    onesc_sb[:].rearrange("p (h o) -> p h o", o=1))

            # ---- phase B: per head QK -> mask/exp -> AV(+denom) ----
            for h in range(NH):
                ti, po = h // 2, (h % 2) * HD
                yps = qq.tile([P, QB], f32, tag="y", name="yps")
                for sb in range(NSB):
                    sps = qq.tile([P, QB], f32, tag="S", name="sps")
                    nc.tensor.matmul(
                        sps[:],
                        r(vT_sb[ti][po:po + HD, sb * P:(sb + 1) * P]),
                        r(vT_sb[ti][po:po + HD, 0:QB]),
                        start=True, stop=True,
                    )
                    e = sp.tile([P, QB], f32r, tag="e", name="e")
                    if sb < 4:
                        # diagonal slot: per-element causal mask, applied
                        # after the w-scale so w==0 keys stay masked
                        s2 = sp.tile([P, QB], f32, tag="s2", name="s2")
                        nc.vector.scalar_tensor_tensor(
                            s2[:], sps[:], w8_sb[:, sb:sb + 1],
                            tri_sb[:, sb * QB:(sb + 1) * QB],
                            ALU.mult, ALU.add,
                        )
                        nc.scalar.activation(
                            e[:], s2[:], AF.Exp,
                            bias=kb_sb[:, sb:sb + 1], scale=1.0)
                    else:
                        nc.scalar.activation(
                            e[:], sps[:], AF.Exp,
                            bias=kb_sb[:, sb:sb + 1],
                            scale=w8_sb[:, sb:sb + 1])
                    nc.tensor.matmul(
                        yps[0:HD + 1, :],
                        r(va_sb[sb][:, h * (HD + 1):(h + 1) * (HD + 1)]),
                        r(e[:]),
                        start=(sb == 0), stop=(sb == NSB - 1),
                    )
                # replicate denominator across 64 partitions, reciprocal,
                # normalize into ySB at this head's row range
                dr = sp.tile([1, QB], f32r, tag="dr", name="dr")
                nc.vector.tensor_copy(dr[:], yps[HD:HD + 1, :])
                dps = qq.tile([P, QB], f32, tag="S", name="sps")
                nc.tensor.matmul(
                    dps[0:HD, :], r(ones_sb[:, 0:HD]), r(dr[:]),
                    start=True, stop=True,
                )
                rec = sp.tile([HD, QB], f32, tag="rec", name="rec")
                nc.vector.reciprocal(rec[:], dps[0:HD, :])
                nc.vector.tensor_mul(
                    y_sb[ti][po:po + HD, :], yps[0:HD, :], rec[:])

            # ---- phase C: out^T = Wp @ y^T + bp ----
            for i in range(4):
                ops = qq.tile([P, QB], f32, tag="vps", name="vps")
                for k in range(4):
                    nc.tensor.matmul(
                        ops[:],
                        r(wpt_sb[k][:, i * P:(i + 1) * P]),
                        r(y_sb[k][:]),
                        start=(k == 0), stop=(k == 3),
                    )
                ot = sp.tile([P, QB], f32, tag="ot", name="ot")
                nc.vector.tensor_scalar_add(ot[:], ops[:], bpp_sb[:, i:i + 1])
                nc.sync.dma_start(out=outT[i * P:(i + 1) * P, :], in_=ot[:])

    _split_multi_waits(nc, mybir)
    return nc


def _get_nc():
    if "nc" not in _cache:
        _cache["nc"] = _build_nc()
    return _cache["nc"]


def _make_in_maps(x, weight, Wv, bv, Wp, bp, state):
    x = np.asarray(x, np.float32)
    w = np.asarray(weight, np.float32)[:, :, 0]
    if not int(np.asarray(state)):
        w = np.ones_like(w)
    WvT = np.ascontiguousarray(np.asarray(Wv, np.float32).T)
    WpT = np.ascontiguousarray(np.asarray(Wp, np.float32).T)
    bv = np.asarray(bv, np.float32)
    bp = np.asarray(bp, np.float32)
    scale = 1.0 / np.sqrt(HD)

    in_maps = []
    for core in range(8):
        b, p = core // 4, core % 4
        nreal = QB * (p + 1)
        perm = np.concatenate(
            [np.arange(QB * p, QB * (p + 1)), np.arange(0, QB * p)])
        kx = np.zeros((T, C), np.float32)
        kx[:nreal] = x[b][perm]
        wp_ = np.zeros((T,), np.float32)
        wp_[:nreal] = w[b][perm]
        kb = np.zeros((T, 1), np.float32)
        kb[nreal:] = NEG
        in_maps.append({
            "kxT": np.ascontiguousarray(kx.T),
            "wvt": WvT,
            "wpt": WpT,
            "bvp": bv.reshape(C, 1),
            "bvr": bv.reshape(1, C),
            "bpp": bp.reshape(C, 1),
            "wvec": wp_.reshape(T, 1).copy(),
            "w8vec": (wp_ * scale).reshape(T, 1).copy(),
            "kbvec": kb,
            "onesr": np.ones((1, P), np.float32),
            "onescol": np.ones((P, NH), np.float32),
        })
    return in_maps


def _gather(results, x):
    out = np.empty((B, T, C), np.float32)
    for core in range(8):
        b, p = core // 4, core % 4
        out[b, QB * p:QB * (p + 1), :] = results[core]["outT"].T
    return out


def _run(in_maps, **kw):
    from concourse.bass_utils import run_bass_kernel_spmd
    return run_bass_kernel_spmd(_get_nc(), in_maps, list(range(8)), **kw)


def kernel(x, weight, Wv, bv, Wp, bp, state):
    in_maps = _make_in_maps(x, weight, Wv, bv, Wp, bp, state)
    res = _run(in_maps)
    return _gather(res.results, x)
